# revision 33
# baseline (speedup 1.0000x reference)
"""AttnBlock (GroupNorm -> single-head self-attention -> residual) on 8 TRN2 cores.

Sharding: B=4 batch elements x 2 query-token halves = 8 cores (SPMD, no
collectives).  Each core receives the full (rolled) channel-major batch
element x^T [C=256, HW=4096], computes GroupNorm + k/v for all 4096
tokens, and q/scores/attention/out-proj for its 2048-token half.  Odd
cores get x rolled by -2048 tokens; attention is permutation-invariant
over keys, so their first 2048 tokens are the original tokens 2048:4096.

Layout is channel-major throughout (tokens on the free axis), which makes
every matmul transpose-free:
  hs^T = GN(x^T)                          [C, N]
  q^T = Wq^T.T @ hs^T  (lhsT=Wq^T)        [C, NQ]
  k^T likewise                            [C, N]
  v   = hs^T.T @ Wv^T  (lhsT=hs^T)        [N, C]   (row-major)
  S^T = k^T.T @ q^T    (lhsT=k^T)         [N, NQ]  (keys on partitions)
  P^T = exp(S^T/16)    (bf16)             softmax numerator, no max-sub
  Z   = ones.T @ P^T   (M=1 matmul)       [1, NQ]  denominators
  o^T = v.T @ P^T      (lhsT=v)           [C, NQ]; scaled by 1/Z
  out^T = Wo^T.T @ o^T + bo               [C, NQ]
  final = (x^T + out^T) / sqrt(2)
"""

import numpy as np
import ml_dtypes

import concourse.bass as bass
import concourse.tile as tile
from concourse import bacc, mybir
from concourse.bass_utils import run_bass_kernel_spmd

dt = mybir.dt
F32, F32R, BF16 = dt.float32, dt.float32r, dt.bfloat16
F8 = dt.float8e4
AF = mybir.ActivationFunctionType
ALU = mybir.AluOpType
DR = mybir.MatmulPerfMode.DoubleRow

P = 128          # partitions
C = 256          # channels
N = 4096         # tokens per batch element (64*64)
NQ = 2048        # query tokens per core
NSTRIP = 512     # query-token strip width (DR matmul: 1024 moving rows)
NS = NQ // NSTRIP  # 4 strips
MT = N // P      # 32 key m-tiles
GS = 8           # channels per group (256 / 32 groups)
EPS = 1e-6
ISCALE = 1.0 / 16.0      # attention scale c**-0.5
ESHIFT = -1.0            # exp shift: keeps exp(s/16-1) < 240 (fp8e4 max); cancels in p=es/Z
RS2 = float(2.0 ** -0.5)  # output residual scale

_prog_cache = {}


def _build_nc():
    nc = bacc.Bacc("TRN2", target_bir_lowering=False, debug=False, num_devices=8)

    def inp(name, shape, d=F32):
        return nc.dram_tensor(name, shape, d, kind="ExternalInput").ap()

    xt_d = inp("xt", [2, P, N])            # [c_half, c_in, n]
    wq_d = inp("wqT", [2, P, C], F8)       # [ci_half, ci_in, c_out] = Wq.T (fp8)
    wk_d = inp("wkT", [2, P, C], F8)
    wv_d = inp("wvT", [2, P, C], F8)
    wo_d = inp("woT", [2, P, C])
    bq_d = inp("bqp", [P, 2])              # [c_out_in, c_out_half]
    bk_d = inp("bkp", [P, 2])
    bos_d = inp("bosp", [P, 2])            # bo * 2^-0.5, packed
    bv_d = inp("bv", [1, C])
    gnw_d = inp("gnw", [P, 2])
    gnb_d = inp("gnb", [P, 2])
    amat_d = inp("amat", [P, P])           # block-diag 8x8 of 1/8
    ones1_d = inp("ones1", [1, P])
    ones8_d = inp("ones8", [P, 2, 32], F8)  # fp8 ones for DoubleRow Z matmul
    out_d = nc.dram_tensor("out", [2, P, NQ], F32, kind="ExternalOutput").ap()

    with tile.TileContext(nc) as tc:
        with (
            tc.tile_pool(name="singles", bufs=1) as singles,
            tc.tile_pool(name="xpool", bufs=1) as xpool,
            tc.tile_pool(name="hsfin", bufs=1) as hsfin,
            tc.tile_pool(name="qk", bufs=1) as qk,
            tc.tile_pool(name="vpool", bufs=1) as vpool,
            tc.tile_pool(name="espool", bufs=2) as espool,
            tc.tile_pool(name="opool", bufs=1) as opool,
            tc.tile_pool(name="small", bufs=2) as small,
            tc.tile_pool(name="zf", bufs=2) as zfpool,
            tc.tile_pool(name="ps", bufs=2, space="PSUM") as ps,
            tc.tile_pool(name="po", bufs=3, space="PSUM") as po,
            tc.tile_pool(name="pz", bufs=1, space="PSUM") as pz,
        ):
            # ---- x load first: keep the HWDGE queues free of weight
            # traffic so GroupNorm stats start as soon as chunks land ----
# ---- x load (chunked; bn_stats pipelined behind each chunk) ----
            xt0 = xpool.tile([P, N], F32, tag="xt0")
            xt1 = xpool.tile([P, N], F32, tag="xt1")
            xts = (xt0, xt1)
            _dmae = [nc.sync, nc.scalar]
            for t in range(2):
                for h in range(4):
                    _dmae[h % 2].dma_start(
                        xts[t][:, h * 1024:(h + 1) * 1024],
                        xt_d[t, :, h * 1024:(h + 1) * 1024])

            # ---- constants / weights (q/k/v fp8 for DoubleRow) ----
            wq = singles.tile([P, 2, C], F8)
            for _ko in range(2):
                nc.gpsimd.dma_start(wq[:, _ko, :], wq_d[_ko])
            wk = singles.tile([P, 2, C], F8)
            for _ko in range(2):
                nc.gpsimd.dma_start(wk[:, _ko, :], wk_d[_ko])
            wv = singles.tile([P, 2, C], F8)
            for _ko in range(2):
                nc.gpsimd.dma_start(wv[:, _ko, :], wv_d[_ko])
            wo = singles.tile([P, 2, C], F32R)
            for _ko in range(2):
                nc.gpsimd.dma_start(wo[:, _ko, :], wo_d[_ko].bitcast(F32R))
            bq = singles.tile([P, 2], F32)
            nc.gpsimd.dma_start(bq[:], bq_d)
            bk = singles.tile([P, 2], F32)
            nc.gpsimd.dma_start(bk[:], bk_d)
            bos = singles.tile([P, 2], F32)
            nc.gpsimd.dma_start(bos[:], bos_d)
            gnw = singles.tile([P, 2], F32)
            nc.gpsimd.dma_start(gnw[:], gnw_d)
            gnb = singles.tile([P, 2], F32)
            nc.gpsimd.dma_start(gnb[:], gnb_d)
            amat = singles.tile([P, P], F32R)
            nc.gpsimd.dma_start(amat[:], amat_d.bitcast(F32R))
            ones1 = singles.tile([1, P], F32R)
            nc.gpsimd.dma_start(ones1[:], ones1_d.bitcast(F32R))
            ones8 = singles.tile([P, 2, 32], F8)
            nc.gpsimd.dma_start(ones8[:], ones8_d)
            # bv broadcast to all partitions (stride-0 partition DMA), 4 copies
            # so the grouped v bias-add covers 4 m-tiles per op
            bvrep4 = singles.tile([P, 4, C], F32)
            bv_b = bass.AP(tensor=bv_d.tensor, offset=bv_d.offset,
                           ap=[[0, P], bv_d.ap[1]])
            for _i in range(4):
                nc.gpsimd.dma_start(out=bvrep4[:, _i, :], in_=bv_b)
            epsap = singles.tile([P, 1], F32)
            nc.vector.memset(epsap[:], EPS)
            eshap = singles.tile([P, 1], F32)
            nc.vector.memset(eshap[:], ESHIFT)

            # ---- GroupNorm (channel-major; stats per channel then 8-chan groups) ----
            hs = hsfin.tile([P, 2, N], F8, tag="hsfin")
            for t in range(2):
                st = small.tile([P, 8, 6], F32, tag="gnst")
                xre = xts[t][:, :].rearrange("p (s f) -> p s f", f=512)
                for sg in range(8):
                    nc.vector.bn_stats(st[:, sg, :], xre[:, sg, :])
                mv = small.tile([P, 2], F32, tag="gnmv")
                nc.vector.bn_aggr(mv[:], st[:])
                # stats2 = [mu, E[x^2]] per channel, rounded to f32r for the matmul
                musq = small.tile([P, 1], F32, tag="gnmusq")
                nc.vector.tensor_mul(musq[:], mv[:, 0:1], mv[:, 0:1])
                stats2 = small.tile([P, 2], F32R, tag="gnst2")
                nc.vector.tensor_copy(stats2[:, 0:1], mv[:, 0:1])
                nc.vector.tensor_add(stats2[:, 1:2], mv[:, 1:2], musq[:])
                # group-aggregate (mean over 8 channels) and broadcast back
                gpt = ps.tile([P, 4, 256], F32, tag="ps", name=f"gp{t}")
                gp = gpt[:, 0, :]
                nc.tensor.matmul(gp[:, 0:2], amat[:], stats2[:], start=True, stop=True)
                gs = small.tile([P, 2], F32, tag="gnagg")
                nc.vector.tensor_copy(gs[:], gp[:, 0:2])
                gvar = small.tile([P, 1], F32, tag="gnvar")
                gmusq = small.tile([P, 1], F32, tag="gnmusq2")
                nc.vector.tensor_mul(gmusq[:], gs[:, 0:1], gs[:, 0:1])
                nc.vector.tensor_tensor(gvar[:], gs[:, 1:2], gmusq[:], ALU.subtract)
                # rstd = exp(-0.5 * ln(var + eps))  (same ACT table set as softmax exp)
                lnv = small.tile([P, 1], F32, tag="gnln")
                nc.scalar.activation(lnv[:], gvar[:], AF.Ln, bias=epsap[:], scale=1.0)
                rstd = small.tile([P, 1], F32, tag="gnrstd")
                nc.scalar.activation(rstd[:], lnv[:], AF.Exp, bias=0.0, scale=-0.5)
                alpha = small.tile([P, 1], F32, tag="gnalpha")
                nc.vector.tensor_mul(alpha[:], rstd[:], gnw[:, t:t + 1])
                atmp = small.tile([P, 1], F32, tag="gnatmp")
                nc.vector.tensor_mul(atmp[:], gs[:, 0:1], alpha[:])
                beta = small.tile([P, 1], F32, tag="gnbeta")
                nc.vector.tensor_tensor(beta[:], gnb[:, t:t + 1], atmp[:], ALU.subtract)
                # apply split across ACT and DVE halves (both write fp8 hs)
                nc.scalar.activation(hs[:, t, 0:2048], xts[t][:, 0:2048],
                                     AF.Identity, bias=beta[:], scale=alpha[:])
                nc.vector.tensor_scalar(hs[:, t, 2048:4096],
                                        xts[t][:, 2048:4096],
                                        alpha[:], beta[:], ALU.mult, ALU.add)

            # ---- q/k projections (fp8 DoubleRow, 512-token blocks) ----
            qT = qk.tile([P, 2, NQ], F8, tag="qT")
            kT = qk.tile([P, 2, N], F8, tag="kT")
            for (wt, bt, dst, nblk) in ((wq, bq, qT, NQ // 512), (wk, bk, kT, N // 512)):
                for ch in range(2):
                    for j in range(nblk // 2):
                        sp = ps.tile([P, 2, 512], F32, tag="ps")
                        for i in range(2):
                            b = 2 * j + i
                            nc.tensor.matmul(
                                sp[:, i, :],
                                wt[:, :, ch * P:(ch + 1) * P],
                                hs[:, :, b * 512:(b + 1) * 512],
                                start=True, stop=True, perf_mode=DR)
                        nc.vector.tensor_scalar(
                            dst[:, ch, 2 * j * 512:(2 * j + 2) * 512],
                            sp[:, 0:2, :].rearrange("p a b -> p (a b)"),
                            bt[:, ch:ch + 1], None, ALU.add)

            # ---- attention strips (v interleaved into strip-0 scores so PE
            # fills the exp-paced phase and the first exp starts early) ----
            v = vpool.tile([P, MT, C], F8)
            final = hsfin.tile([P, 2, NQ], F32, tag="hsfin")
            for s in range(NS):
                ns = slice(s * NSTRIP, (s + 1) * NSTRIP)
                es = espool.tile([P, MT, NSTRIP], F8, tag="es")
                for g in range(MT // 2):
                    sp = ps.tile([P, 2, 512], F32, tag="ps")
                    for i in range(2):
                        m = 2 * g + i
                        nc.tensor.matmul(sp[:, i, :], kT[:, :, m * P:(m + 1) * P],
                                         qT[:, :, ns], start=True, stop=True,
                                         perf_mode=DR)
                    nc.scalar.activation(es[:, 2 * g:2 * g + 2, :], sp[:],
                                         AF.Exp, bias=eshap[:], scale=ISCALE)
                    if s == 0 and g < MT // 4:
                        # v projection (fp8 DR), 4 m-tiles per psum tile
                        vp = ps.tile([P, 4, 256], F32, tag="ps", name=f"vp{g}")
                        for i in range(4):
                            m = 4 * g + i
                            nc.tensor.matmul(vp[:, i, :], hs[:, :, m * P:(m + 1) * P],
                                             wv[:, :, :], start=True, stop=True,
                                             perf_mode=DR)
                        nc.vector.tensor_add(v[:, 4 * g:4 * g + 4, :], vp[:],
                                             bvrep4[:])
                # softmax denominators: Z = ones.T @ P^T, one DR chain
                # (DoubleRow forbids tile_position != (0,0); 32 identical rows)
                zp = pz.tile([P, NSTRIP], F32, tag="pz")
                for t in range(MT // 2):
                    nc.tensor.matmul(zp[0:32, :], ones8[:],
                                     es[:, 2 * t:2 * t + 2, :],
                                     start=(t == 0), stop=(t == MT // 2 - 1),
                                     perf_mode=DR)
                # attn @ v (per-channel-half chains; separate 1-bank psum tiles)
                ops = []
                for ch in range(2):
                    opc = po.tile([P, NSTRIP], F32, tag="po", name=f"op{ch}_{s}")
                    ops.append(opc)
                    for t in range(MT // 2):
                        nc.tensor.matmul(opc[:],
                                         v[:, 2 * t:2 * t + 2, ch * P:(ch + 1) * P],
                                         es[:, 2 * t:2 * t + 2, :],
                                         start=(t == 0), stop=(t == MT // 2 - 1),
                                         perf_mode=DR)
                # 1/Z on DVE (avoids ACT table-set thrash), broadcast via K=1 matmul
                rz = small.tile([1, NSTRIP], F32R, tag="rz")
                with nc.allow_low_precision(reason="f32r rounding of 1/Z"):
                    nc.vector.reciprocal(rz[:], zp[0:1, :])
                rp = po.tile([P, NSTRIP], F32, tag="po", name=f"rp{s}")
                nc.tensor.matmul(rp[:], ones1[:], rz[:], start=True, stop=True)
                rzs = small.tile([P, NSTRIP], F32, tag="rzs")
                nc.vector.tensor_copy(rzs[:], rp[:])
                o = opool.tile([P, 2, NQ], F32R, tag="o")
                for ch in range(2):
                    nc.vector.tensor_mul(o[:, ch, ns], ops[ch][:], rzs[:])
                # out projection + bias + residual + 2^-0.5
                z2 = zfpool.tile([P, 2, NSTRIP], F32, tag="zf")
                for ch in range(2):
                    op2c = po.tile([P, NSTRIP], F32, tag="po", name=f"op2{ch}_{s}")
                    for ko in range(2):
                        nc.tensor.matmul(op2c[:],
                                         wo[:, ko, ch * P:(ch + 1) * P],
                                         o[:, ko, ns], start=(ko == 0), stop=(ko == 1))
                    nc.vector.tensor_scalar(z2[:, ch, :], op2c[:],
                                            RS2, bos[:, ch:ch + 1],
                                            ALU.mult, ALU.add)
                for t in range(2):
                    nc.vector.scalar_tensor_tensor(
                        out=final[:, t, ns], in0=xts[t][:, ns], scalar=RS2,
                        in1=z2[:, t, :], op0=ALU.mult, op1=ALU.add)
                    nc.sync.dma_start(out_d[t, :, ns], final[:, t, ns])

    nc.finalize()
    return nc


def _get_nc():
    if "nc" not in _prog_cache:
        _prog_cache["nc"] = _build_nc()
    return _prog_cache["nc"]


def _make_in_maps(x, gn_weight, gn_bias, Wq, bq, Wk, bk, Wv, bv, Wo, bo):
    x = np.asarray(x, dtype=np.float32)
    f32 = lambda a: np.ascontiguousarray(np.asarray(a, dtype=np.float32))

    def packT(b_vec):  # [256] -> [128, 2] (c_out_in, c_out_half)
        return np.ascontiguousarray(f32(b_vec).reshape(2, P).T)

    amat = np.zeros((P, P), np.float32)
    for g in range(P // GS):
        amat[g * GS:(g + 1) * GS, g * GS:(g + 1) * GS] = 1.0 / GS

    f8 = lambda a: np.ascontiguousarray(
        np.asarray(a, dtype=np.float32).astype(ml_dtypes.float8_e4m3))
    common = {
        "wqT": f8(np.asarray(Wq).T).reshape(2, P, C),
        "wkT": f8(np.asarray(Wk).T).reshape(2, P, C),
        "wvT": f8(np.asarray(Wv).T).reshape(2, P, C),
        "woT": f32(np.asarray(Wo).T).reshape(2, P, C),
        "bqp": packT(bq),
        "bkp": packT(bk),
        "bosp": packT(np.asarray(bo, dtype=np.float32) * RS2),
        "bv": f32(bv).reshape(1, C),
        "gnw": packT(gn_weight),
        "gnb": packT(gn_bias),
        "amat": amat,
        "ones1": np.ones((1, P), np.float32),
        "ones8": np.ones((P, 2, 32), ml_dtypes.float8_e4m3),
    }

    in_maps = []
    for core in range(8):
        b, half = core // 2, core % 2
        xt = x[b].reshape(C, N)
        if half:
            xt = np.roll(xt, -NQ, axis=1)
        in_maps.append({"xt": np.ascontiguousarray(xt).reshape(2, P, N), **common})
    return in_maps


def _assemble(results, B):
    out = np.empty((B, C, N), np.float32)
    for core in range(2 * B):
        b, half = core // 2, core % 2
        out[b, :, half * NQ:(half + 1) * NQ] = results[core]["out"].reshape(C, NQ)
    return out.reshape(B, C, 64, 64)


def kernel(x, gn_weight, gn_bias, Wq, bq, Wk, bk, Wv, bv, Wo, bo):
    x = np.asarray(x, dtype=np.float32)
    in_maps = _make_in_maps(x, gn_weight, gn_bias, Wq, bq, Wk, bk, Wv, bv, Wo, bo)
    nc = _get_nc()
    res = run_bass_kernel_spmd(nc, in_maps, list(range(8)))
    return _assemble(res.results, x.shape[0])



# revision 37
# speedup vs baseline: 1.1879x; 1.1879x over previous
"""AttnBlock (GroupNorm -> single-head self-attention -> residual) on 8 TRN2 cores.

Sharding: B=4 batch elements x 2 query-token halves = 8 cores (SPMD, no
collectives).  Each core receives the full (rolled) channel-major batch
element x^T [C=256, HW=4096], computes GroupNorm + k/v for all 4096
tokens, and q/scores/attention/out-proj for its 2048-token half.  Odd
cores get x rolled by -2048 tokens; attention is permutation-invariant
over keys, so their first 2048 tokens are the original tokens 2048:4096.

Layout is channel-major throughout (tokens on the free axis), which makes
every matmul transpose-free:
  hs^T = GN(x^T)                          [C, N]
  q^T = Wq^T.T @ hs^T  (lhsT=Wq^T)        [C, NQ]
  k^T likewise                            [C, N]
  v   = hs^T.T @ Wv^T  (lhsT=hs^T)        [N, C]   (row-major)
  S^T = k^T.T @ q^T    (lhsT=k^T)         [N, NQ]  (keys on partitions)
  P^T = exp(S^T/16)    (bf16)             softmax numerator, no max-sub
  Z   = ones.T @ P^T   (M=1 matmul)       [1, NQ]  denominators
  o^T = v.T @ P^T      (lhsT=v)           [C, NQ]; scaled by 1/Z
  out^T = Wo^T.T @ o^T + bo               [C, NQ]
  final = (x^T + out^T) / sqrt(2)
"""

import numpy as np
import ml_dtypes

import concourse.bass as bass
import concourse.tile as tile
from concourse import bacc, mybir
from concourse.bass_utils import run_bass_kernel_spmd

dt = mybir.dt
F32, F32R, BF16 = dt.float32, dt.float32r, dt.bfloat16
F8 = dt.float8e4
AF = mybir.ActivationFunctionType
ALU = mybir.AluOpType
DR = mybir.MatmulPerfMode.DoubleRow

P = 128          # partitions
C = 256          # channels
N = 4096         # tokens per batch element (64*64)
NQ = 2048        # query tokens per core
NSTRIP = 256     # query-token strip width (DR 2x only holds up to 512 moving rows)
NS = NQ // NSTRIP  # 8 strips
MT = N // P      # 32 key m-tiles
GS = 8           # channels per group (256 / 32 groups)
EPS = 1e-6
ISCALE = 1.0 / 16.0      # attention scale c**-0.5
ESHIFT = -1.0            # exp shift: keeps exp(s/16-1) < 240 (fp8e4 max); cancels in p=es/Z
RS2 = float(2.0 ** -0.5)  # output residual scale

_prog_cache = {}


def _build_nc():
    nc = bacc.Bacc("TRN2", target_bir_lowering=False, debug=False, num_devices=8)

    def inp(name, shape, d=F32):
        return nc.dram_tensor(name, shape, d, kind="ExternalInput").ap()

    xt_d = inp("xt", [2, P, N])            # [c_half, c_in, n]
    wq_d = inp("wqT", [2, P, C], F8)       # [ci_half, ci_in, c_out] = Wq.T (fp8)
    wk_d = inp("wkT", [2, P, C], F8)
    wv_d = inp("wvT", [2, P, C], F8)
    wo_d = inp("woT", [2, P, C])
    bq_d = inp("bqp", [P, 2])              # [c_out_in, c_out_half]
    bk_d = inp("bkp", [P, 2])
    bos_d = inp("bosp", [P, 2])            # bo * 2^-0.5, packed
    bv_d = inp("bv", [1, C])
    gnw_d = inp("gnw", [P, 2])
    gnb_d = inp("gnb", [P, 2])
    amat_d = inp("amat", [P, P])           # block-diag 8x8 of 1/8
    ones1_d = inp("ones1", [1, P])
    ones8_d = inp("ones8", [P, 2, 32], F8)  # fp8 ones for DoubleRow Z matmul
    out_d = nc.dram_tensor("out", [2, P, NQ], F32, kind="ExternalOutput").ap()

    with tile.TileContext(nc) as tc:
        with (
            tc.tile_pool(name="singles", bufs=1) as singles,
            tc.tile_pool(name="xpool", bufs=1) as xpool,
            tc.tile_pool(name="hsfin", bufs=1) as hsfin,
            tc.tile_pool(name="qk", bufs=1) as qk,
            tc.tile_pool(name="vpool", bufs=1) as vpool,
            tc.tile_pool(name="espool", bufs=2) as espool,
            tc.tile_pool(name="opool", bufs=1) as opool,
            tc.tile_pool(name="small", bufs=2) as small,
            tc.tile_pool(name="zf", bufs=2) as zfpool,
            tc.tile_pool(name="ps", bufs=2, space="PSUM") as ps,
            tc.tile_pool(name="po", bufs=2, space="PSUM") as po,
            tc.tile_pool(name="pm", bufs=1, space="PSUM") as pm,
            tc.tile_pool(name="pz", bufs=1, space="PSUM") as pz,
        ):
            # ---- x load first: keep the HWDGE queues free of weight
            # traffic so GroupNorm stats start as soon as chunks land ----
# ---- x load (chunked; bn_stats pipelined behind each chunk) ----
            xt0 = xpool.tile([P, N], F32, tag="xt0")
            xt1 = xpool.tile([P, N], F32, tag="xt1")
            xts = (xt0, xt1)
            _dmae = [nc.sync, nc.scalar]
            for t in range(2):
                for h in range(4):
                    _dmae[h % 2].dma_start(
                        xts[t][:, h * 1024:(h + 1) * 1024],
                        xt_d[t, :, h * 1024:(h + 1) * 1024])

            # ---- constants / weights (q/k/v fp8 for DoubleRow) ----
            wq = singles.tile([P, 2, C], F8)
            for _ko in range(2):
                nc.gpsimd.dma_start(wq[:, _ko, :], wq_d[_ko])
            wk = singles.tile([P, 2, C], F8)
            for _ko in range(2):
                nc.gpsimd.dma_start(wk[:, _ko, :], wk_d[_ko])
            wv = singles.tile([P, 2, C], F8)
            for _ko in range(2):
                nc.gpsimd.dma_start(wv[:, _ko, :], wv_d[_ko])
            wo = singles.tile([P, 2, C], F32R)
            for _ko in range(2):
                nc.gpsimd.dma_start(wo[:, _ko, :], wo_d[_ko].bitcast(F32R))
            bq = singles.tile([P, 2], F32)
            nc.gpsimd.dma_start(bq[:], bq_d)
            bk = singles.tile([P, 2], F32)
            nc.gpsimd.dma_start(bk[:], bk_d)
            bos = singles.tile([P, 2], F32)
            nc.gpsimd.dma_start(bos[:], bos_d)
            gnw = singles.tile([P, 2], F32)
            nc.gpsimd.dma_start(gnw[:], gnw_d)
            gnb = singles.tile([P, 2], F32)
            nc.gpsimd.dma_start(gnb[:], gnb_d)
            amat = singles.tile([P, P], F32R)
            nc.gpsimd.dma_start(amat[:], amat_d.bitcast(F32R))
            ones1 = singles.tile([1, P], F32R)
            nc.gpsimd.dma_start(ones1[:], ones1_d.bitcast(F32R))
            ones8 = singles.tile([P, 2, 32], F8)
            nc.gpsimd.dma_start(ones8[:], ones8_d)
            # bv broadcast to all partitions (stride-0 partition DMA), 4 copies
            # so the grouped v bias-add covers 4 m-tiles per op
            bvrep4 = singles.tile([P, 4, C], F32)
            bv_b = bass.AP(tensor=bv_d.tensor, offset=bv_d.offset,
                           ap=[[0, P], bv_d.ap[1]])
            for _i in range(4):
                nc.gpsimd.dma_start(out=bvrep4[:, _i, :], in_=bv_b)
            epsap = singles.tile([P, 1], F32)
            nc.vector.memset(epsap[:], EPS)
            eshap = singles.tile([P, 1], F32)
            nc.vector.memset(eshap[:], ESHIFT)

            # ---- GroupNorm (channel-major; stats per channel then 8-chan groups) ----
            hs = hsfin.tile([P, 2, N], F8, tag="hsfin")
            for t in range(2):
                st = small.tile([P, 8, 6], F32, tag="gnst")
                xre = xts[t][:, :].rearrange("p (s f) -> p s f", f=512)
                for sg in range(8):
                    nc.vector.bn_stats(st[:, sg, :], xre[:, sg, :])
                mv = small.tile([P, 2], F32, tag="gnmv")
                nc.vector.bn_aggr(mv[:], st[:])
                # stats2 = [mu, E[x^2]] per channel, rounded to f32r for the matmul
                musq = small.tile([P, 1], F32, tag="gnmusq")
                nc.vector.tensor_mul(musq[:], mv[:, 0:1], mv[:, 0:1])
                stats2 = small.tile([P, 2], F32R, tag="gnst2")
                nc.vector.tensor_copy(stats2[:, 0:1], mv[:, 0:1])
                nc.vector.tensor_add(stats2[:, 1:2], mv[:, 1:2], musq[:])
                # group-aggregate (mean over 8 channels) and broadcast back
                gp = pm.tile([P, 512], F32, tag="pm")
                nc.tensor.matmul(gp[:, 0:2], amat[:], stats2[:], start=True, stop=True)
                gs = small.tile([P, 2], F32, tag="gnagg")
                nc.vector.tensor_copy(gs[:], gp[:, 0:2])
                gvar = small.tile([P, 1], F32, tag="gnvar")
                gmusq = small.tile([P, 1], F32, tag="gnmusq2")
                nc.vector.tensor_mul(gmusq[:], gs[:, 0:1], gs[:, 0:1])
                nc.vector.tensor_tensor(gvar[:], gs[:, 1:2], gmusq[:], ALU.subtract)
                # rstd = exp(-0.5 * ln(var + eps))  (same ACT table set as softmax exp)
                lnv = small.tile([P, 1], F32, tag="gnln")
                nc.scalar.activation(lnv[:], gvar[:], AF.Ln, bias=epsap[:], scale=1.0)
                rstd = small.tile([P, 1], F32, tag="gnrstd")
                nc.scalar.activation(rstd[:], lnv[:], AF.Exp, bias=0.0, scale=-0.5)
                alpha = small.tile([P, 1], F32, tag="gnalpha")
                nc.vector.tensor_mul(alpha[:], rstd[:], gnw[:, t:t + 1])
                atmp = small.tile([P, 1], F32, tag="gnatmp")
                nc.vector.tensor_mul(atmp[:], gs[:, 0:1], alpha[:])
                beta = small.tile([P, 1], F32, tag="gnbeta")
                nc.vector.tensor_tensor(beta[:], gnb[:, t:t + 1], atmp[:], ALU.subtract)
                # apply split across ACT and DVE halves (both write fp8 hs)
                nc.scalar.activation(hs[:, t, 0:2048], xts[t][:, 0:2048],
                                     AF.Identity, bias=beta[:], scale=alpha[:])
                nc.vector.tensor_scalar(hs[:, t, 2048:4096],
                                        xts[t][:, 2048:4096],
                                        alpha[:], beta[:], ALU.mult, ALU.add)

            # ---- q/k projections (fp8 DoubleRow over the two c_in halves) ----
            qT = qk.tile([P, 2, NQ], F8, tag="qT")
            kT = qk.tile([P, 2, N], F8, tag="kT")
            for (wt, bt, dst, nblk) in ((wq, bq, qT, NQ // 256), (wk, bk, kT, N // 256)):
                for ch in range(2):
                    for j in range(nblk // 2):
                        sp = ps.tile([P, 4, NSTRIP], F32, tag="ps")
                        for i in range(2):
                            b = 2 * j + i
                            nc.tensor.matmul(
                                sp[:, i, :],
                                wt[:, :, ch * P:(ch + 1) * P],
                                hs[:, :, b * 256:(b + 1) * 256],
                                start=True, stop=True, perf_mode=DR)
                        nc.vector.tensor_scalar(
                            dst[:, ch, 2 * j * 256:(2 * j + 2) * 256],
                            sp[:, 0:2, :].rearrange("p a b -> p (a b)"),
                            bt[:, ch:ch + 1], None, ALU.add)

            # ---- attention strips (v interleaved into strip-0 scores so PE
            # fills the exp-paced phase and the first exp starts early) ----
            v = vpool.tile([P, MT, C], F8)
            final = hsfin.tile([P, 2, NQ], F32, tag="hsfin")
            for s in range(NS):
                ns = slice(s * NSTRIP, (s + 1) * NSTRIP)
                es = espool.tile([P, MT, NSTRIP], F8, tag="es")
                for j in range(MT // 4):
                    sp = ps.tile([P, 4, NSTRIP], F32, tag="ps")
                    for i in range(4):
                        m = 4 * j + i
                        nc.tensor.matmul(sp[:, i, :], kT[:, :, m * P:(m + 1) * P],
                                         qT[:, :, ns], start=True, stop=True,
                                         perf_mode=DR)
                    nc.scalar.activation(es[:, 4 * j:4 * j + 4, :], sp[:],
                                         AF.Exp, bias=eshap[:], scale=ISCALE)
                    if s == 0:
                        # v projection (fp8 DR), 4 m-tiles per psum tile;
                        # drafts behind the exp-paced strip-0 phase
                        vp = ps.tile([P, 4, NSTRIP], F32, tag="ps", name=f"vp{j}")
                        for i in range(4):
                            m = 4 * j + i
                            nc.tensor.matmul(vp[:, i, :], hs[:, :, m * P:(m + 1) * P],
                                             wv[:, :, :], start=True, stop=True,
                                             perf_mode=DR)
                        nc.vector.tensor_add(v[:, 4 * j:4 * j + 4, :], vp[:],
                                             bvrep4[:])
                # softmax denominators: Z = ones.T @ P^T, one DR chain
                # (DoubleRow forbids tile_position != (0,0); 32 identical rows)
                zp = pz.tile([P, NSTRIP], F32, tag="pz")
                for t in range(MT // 2):
                    nc.tensor.matmul(zp[0:32, :], ones8[:],
                                     es[:, 2 * t:2 * t + 2, :],
                                     start=(t == 0), stop=(t == MT // 2 - 1),
                                     perf_mode=DR)
                # attn @ v
                op = po.tile([P, 2, NSTRIP], F32, tag="po")
                for ch in range(2):
                    for t in range(MT // 2):
                        nc.tensor.matmul(op[:, ch, :],
                                         v[:, 2 * t:2 * t + 2, ch * P:(ch + 1) * P],
                                         es[:, 2 * t:2 * t + 2, :],
                                         start=(t == 0), stop=(t == MT // 2 - 1),
                                         perf_mode=DR)
                # 1/Z on DVE (avoids ACT table-set thrash), broadcast via K=1 matmul
                rz = small.tile([1, NSTRIP], F32R, tag="rz")
                with nc.allow_low_precision(reason="f32r rounding of 1/Z"):
                    nc.vector.reciprocal(rz[:], zp[0:1, :])
                rp = pm.tile([P, 512], F32, tag="pm")
                nc.tensor.matmul(rp[:, 0:NSTRIP], ones1[:], rz[:], start=True, stop=True)
                rzs = small.tile([P, NSTRIP], F32, tag="rzs")
                nc.vector.tensor_copy(rzs[:], rp[:, 0:NSTRIP])
                o = opool.tile([P, 2, NQ], F32R, tag="o")
                for ch in range(2):
                    nc.vector.tensor_mul(o[:, ch, ns], op[:, ch, :], rzs[:])
                # out projection + bias + residual + 2^-0.5
                op2 = po.tile([P, 2, NSTRIP], F32, tag="po", name=f"op2_{s}")
                z2 = zfpool.tile([P, 2, NSTRIP], F32, tag="zf")
                for ch in range(2):
                    for ko in range(2):
                        nc.tensor.matmul(op2[:, ch, :],
                                         wo[:, ko, ch * P:(ch + 1) * P],
                                         o[:, ko, ns], start=(ko == 0), stop=(ko == 1))
                    nc.vector.tensor_scalar(z2[:, ch, :], op2[:, ch, :],
                                            RS2, bos[:, ch:ch + 1],
                                            ALU.mult, ALU.add)
                for t in range(2):
                    nc.vector.scalar_tensor_tensor(
                        out=final[:, t, ns], in0=xts[t][:, ns], scalar=RS2,
                        in1=z2[:, t, :], op0=ALU.mult, op1=ALU.add)
                    nc.sync.dma_start(out_d[t, :, ns], final[:, t, ns])

    nc.finalize()
    return nc


def _get_nc():
    if "nc" not in _prog_cache:
        _prog_cache["nc"] = _build_nc()
    return _prog_cache["nc"]


def _make_in_maps(x, gn_weight, gn_bias, Wq, bq, Wk, bk, Wv, bv, Wo, bo):
    x = np.asarray(x, dtype=np.float32)
    f32 = lambda a: np.ascontiguousarray(np.asarray(a, dtype=np.float32))

    def packT(b_vec):  # [256] -> [128, 2] (c_out_in, c_out_half)
        return np.ascontiguousarray(f32(b_vec).reshape(2, P).T)

    amat = np.zeros((P, P), np.float32)
    for g in range(P // GS):
        amat[g * GS:(g + 1) * GS, g * GS:(g + 1) * GS] = 1.0 / GS

    f8 = lambda a: np.ascontiguousarray(
        np.asarray(a, dtype=np.float32).astype(ml_dtypes.float8_e4m3))
    common = {
        "wqT": f8(np.asarray(Wq).T).reshape(2, P, C),
        "wkT": f8(np.asarray(Wk).T).reshape(2, P, C),
        "wvT": f8(np.asarray(Wv).T).reshape(2, P, C),
        "woT": f32(np.asarray(Wo).T).reshape(2, P, C),
        "bqp": packT(bq),
        "bkp": packT(bk),
        "bosp": packT(np.asarray(bo, dtype=np.float32) * RS2),
        "bv": f32(bv).reshape(1, C),
        "gnw": packT(gn_weight),
        "gnb": packT(gn_bias),
        "amat": amat,
        "ones1": np.ones((1, P), np.float32),
        "ones8": np.ones((P, 2, 32), ml_dtypes.float8_e4m3),
    }

    in_maps = []
    for core in range(8):
        b, half = core // 2, core % 2
        xt = x[b].reshape(C, N)
        if half:
            xt = np.roll(xt, -NQ, axis=1)
        in_maps.append({"xt": np.ascontiguousarray(xt).reshape(2, P, N), **common})
    return in_maps


def _assemble(results, B):
    out = np.empty((B, C, N), np.float32)
    for core in range(2 * B):
        b, half = core // 2, core % 2
        out[b, :, half * NQ:(half + 1) * NQ] = results[core]["out"].reshape(C, NQ)
    return out.reshape(B, C, 64, 64)


def kernel(x, gn_weight, gn_bias, Wq, bq, Wk, bk, Wv, bv, Wo, bo):
    x = np.asarray(x, dtype=np.float32)
    in_maps = _make_in_maps(x, gn_weight, gn_bias, Wq, bq, Wk, bk, Wv, bv, Wo, bo)
    nc = _get_nc()
    res = run_bass_kernel_spmd(nc, in_maps, list(range(8)))
    return _assemble(res.results, x.shape[0])

